# revision 14
# baseline (speedup 1.0000x reference)
"""DeepseekV3 MoE kernel for 8 Trainium2 NeuronCores (expert-parallel).

Strategy:
  - Host: grouped top-k gating (exact replica of the reference jax ops, on CPU),
    token dispatch (gather tokens per expert, zero-padded to per-slot capacity).
    Experts are assigned to (core, slot) by descending token count so the
    biggest experts land in the biggest slots (same slot sizes on every core —
    the kernel is SPMD).
  - Device (SPMD over 8 cores): core c owns 8 routed expert slots plus a
    (token-half, intermediate-quarter) tile of the two shared experts
    (core c -> tokens [c//4 * 512, +512), intermediate cols [c%4 * 128, +128)).
    Everything is bf16 (weights, activations, outputs); PSUM accumulates fp32.
    The kernel is DMA-bound (~32.5 MB/core at ~358 GB/s), so tensors ship
    partition-major as large DMAs; token buffers and shared constants are
    prefetched; gate_up weights ship as two halves so compute starts before
    the full tensor lands.
  - Host: scale expert rows by combine weight, scatter-add back by token,
    sum shared partials.

Shapes (hardcoded): T=1024, H=1024, I=512, E=64, S=2, G=8, TOPK_GROUP=4, K=8.
"""
import numpy as np
import ml_dtypes
from contextlib import ExitStack

import concourse.bass as bass
from concourse import mybir, tile, bacc
from concourse.bass_utils import run_bass_kernel_spmd

f32 = mybir.dt.float32
bf16 = mybir.dt.bfloat16
AF = mybir.ActivationFunctionType
BF = ml_dtypes.bfloat16

T, H, I, E, S = 1024, 1024, 512, 64, 2
G, TOPK_GROUP, K = 8, 4, 8
I2 = 2 * I
N_CORES = 8
E_LOC = E // N_CORES          # 8 expert slots per core
HT = H // 128                 # 8 k-tiles over hidden dim
IT = I // 128                 # 4 tiles over intermediate dim
TSH = 512                     # shared: tokens per core (T / 2 token groups)
ISHC = 128                    # shared: intermediate cols per core (I / 4)

# Per-slot token capacities. Slot j on every core holds the experts of global
# count-rank 8j..8j+7 (one per core), so cap[j] only needs to cover that
# rank band. key(0) band maxima: [151,143,139,133,129,125,119,114].
CAPS = [156, 148, 144, 138, 134, 130, 124, 118]
XOFF = np.concatenate([[0], np.cumsum([HT * c for c in CAPS])]).astype(int)
XTOT = int(XOFF[-1])

_TRACE = False
_CACHED_NC = None
LAST_RESULTS = None


def _build_nc():
    nc = bacc.Bacc("TRN2", target_bir_lowering=False, debug=False)

    # partition-major layouts: [128, chunk, free] so each partition's DRAM run
    # is contiguous (big DMA packets)
    xg_d = nc.dram_tensor("xg", [128, XTOT], bf16, kind="ExternalInput")
    # gate_up weights as two column-halves: half q = gate cols [q*256,+256) ++
    # up cols [512+q*256,+256), each [128, HT, 512]
    wgu_d = nc.dram_tensor("wgu", [E_LOC, 2, 128, HT, 512], bf16,
                           kind="ExternalInput")
    wd_d = nc.dram_tensor("wd", [E_LOC, 128, IT, H], bf16, kind="ExternalInput")
    xt_d = nc.dram_tensor("xt", [128, HT, TSH], bf16, kind="ExternalInput")
    swh_d = nc.dram_tensor("swh", [S, 128, HT, 2 * ISHC], bf16,
                           kind="ExternalInput")
    sdc_d = nc.dram_tensor("sdc", [128, S, H], bf16, kind="ExternalInput")
    y_d = nc.dram_tensor("y", [128, XTOT], bf16, kind="ExternalOutput")
    sh_d = nc.dram_tensor("sh", [TSH, H], bf16, kind="ExternalOutput")

    with tile.TileContext(nc) as tc, ExitStack() as ctx:
        wgu_p = ctx.enter_context(tc.tile_pool(name="wgu", bufs=3))
        wd_p = ctx.enter_context(tc.tile_pool(name="wd", bufs=3))
        h_p = ctx.enter_context(tc.tile_pool(name="h", bufs=2))
        y_p = ctx.enter_context(tc.tile_pool(name="y", bufs=3))
        const_p = ctx.enter_context(tc.tile_pool(name="const", bufs=1))
        shh_p = ctx.enter_context(tc.tile_pool(name="shh", bufs=2))
        psA = ctx.enter_context(tc.tile_pool(name="psA", bufs=4, space="PSUM"))
        psB = ctx.enter_context(tc.tile_pool(name="psB", bufs=2, space="PSUM"))
        psC = ctx.enter_context(tc.tile_pool(name="psC", bufs=2, space="PSUM"))

        # ---- prefetch: slot-0 and slot-1 inputs first so compute starts
        # immediately, then the remaining token buffers and shared constants.
        xg0 = const_p.tile([128, XOFF[1]], bf16, tag="xg0")
        nc.sync.dma_start(xg0[:], xg_d.ap()[:, 0:XOFF[1]])

        def load_wgu(j):
            wg = [wgu_p.tile([128, HT, 512], bf16, tag=f"wgu{q}",
                             name=f"wg{j}_{q}") for q in range(2)]
            nc.sync.dma_start(wg[0][:], wgu_d.ap()[j, 0])
            nc.sync.dma_start(wg[1][:], wgu_d.ap()[j, 1])
            return wg

        def load_wd(j):
            wd = wd_p.tile([128, IT, H], bf16, tag="wd", name=f"wd{j}")
            nc.sync.dma_start(wd[:], wd_d.ap()[j])
            return wd

        wgs, wds = {}, {}
        wgs[0] = load_wgu(0)
        wds[0] = load_wd(0)
        wgs[1] = load_wgu(1)
        wds[1] = load_wd(1)
        xgr = const_p.tile([128, XTOT - XOFF[1]], bf16, tag="xgr")
        nc.sync.dma_start(xgr[:], xg_d.ap()[:, XOFF[1]:XTOT])
        xt = const_p.tile([128, HT, TSH], bf16, tag="xt")
        nc.sync.dma_start(xt[:], xt_d.ap()[:])
        swh_sb = []
        for s in range(S):
            sw = const_p.tile([128, HT, 2 * ISHC], bf16, tag=f"swh{s}",
                              name=f"swh_t{s}")
            nc.sync.dma_start(sw[:], swh_d.ap()[s])
            swh_sb.append(sw)
        sdc_sb = const_p.tile([128, S, H], bf16, tag="sdc")
        nc.sync.dma_start(sdc_sb[:], sdc_d.ap()[:])

        def xg_ap(j, h, cap):
            # [128, cap] slice of slot j's token buffer, hidden chunk h
            if j == 0:
                return xg0[:, h * cap:(h + 1) * cap]
            base = XOFF[j] - XOFF[1]
            return xgr[:, base + h * cap:base + (h + 1) * cap]

        def emit_expert(j):
            cap = CAPS[j]
            wg, wd = wgs[j], wds[j]

            h_t = []
            for it in range(IT):
                q, l = it // 2, it % 2
                pss = []
                for half in range(2):  # gate, up
                    ps = psA.tile([128, cap], f32, tag="gu")
                    off = half * 256 + l * 128
                    for h in range(HT):
                        nc.tensor.matmul(
                            ps[:], wg[q][:, h, off:off + 128], xg_ap(j, h, cap),
                            start=(h == 0), stop=(h == HT - 1))
                    pss.append(ps)
                sl = h_p.tile([128, cap], f32, tag="silu")
                nc.scalar.activation(sl[:], pss[0][:], AF.Silu)
                hh = h_p.tile([128, cap], bf16, tag=f"h{it}")
                nc.vector.tensor_mul(hh[:], sl[:], pss[1][:])
                h_t.append(hh)

            # The last two slots trail the DMA stream, so deepen their down
            # pipeline (borrow the then-idle shared-expert PSUM pool) and
            # issue their stores from the then-idle sync queue.
            tail = j >= E_LOC - 2
            yo = y_p.tile([128, HT * cap], bf16, tag="y")
            split = j == E_LOC - 1  # halve the last store so its completion
            for hs in range(HT):    # latency overlaps the remaining copies
                if tail and hs % 2 == 1:
                    ps_big = psC.tile([128, 512], f32, tag="big", name="ps_big")
                    ps_y = ps_big[:, 0:cap]
                else:
                    ps_t = psB.tile([128, cap], f32, tag="yps", name="ps_t")
                    ps_y = ps_t[:]
                for it in range(IT):
                    nc.tensor.matmul(ps_y, wd[:, it, hs * 128:(hs + 1) * 128],
                                     h_t[it][:], start=(it == 0), stop=(it == IT - 1))
                if hs % 2 == 0:
                    nc.vector.tensor_copy(yo[:, hs * cap:(hs + 1) * cap], ps_y)
                else:
                    nc.scalar.activation(yo[:, hs * cap:(hs + 1) * cap], ps_y,
                                         AF.Copy)
                if split and hs == HT // 2 - 1:
                    nc.sync.dma_start(
                        y_d.ap()[:, XOFF[j]:XOFF[j] + (HT // 2) * cap],
                        yo[:, :(HT // 2) * cap])
            if split:
                nc.sync.dma_start(y_d.ap()[:, XOFF[j] + (HT // 2) * cap:XOFF[j + 1]],
                                  yo[:, (HT // 2) * cap:])
            elif tail:
                nc.sync.dma_start(y_d.ap()[:, XOFF[j]:XOFF[j + 1]], yo[:])
            else:
                nc.scalar.dma_start(y_d.ap()[:, XOFF[j]:XOFF[j + 1]], yo[:])

        hc_t = []

        def emit_shared_gu():
            # hc_t[s] = silu(gate)*up for shared expert s: [128 i-cols, TSH]
            for s in range(S):
                pss = []
                for half in range(2):  # gate, up
                    ps = psC.tile([128, TSH], f32, tag="big")
                    for h in range(HT):
                        nc.tensor.matmul(ps[:],
                                         swh_sb[s][:, h, half * ISHC:(half + 1) * ISHC],
                                         xt[:, h, :],
                                         start=(h == 0), stop=(h == HT - 1))
                    pss.append(ps)
                sl = shh_p.tile([128, TSH], f32, tag="slsh")
                nc.scalar.activation(sl[:], pss[0][:], AF.Silu)
                hc = shh_p.tile([128, TSH], bf16, tag=f"hc{s}", name=f"hc_t{s}")
                nc.vector.tensor_mul(hc[:], sl[:], pss[1][:])
                hc_t.append(hc)

        def emit_shared_down():
            for tp in range(4):
                so = y_p.tile([128, H], bf16, tag="sh")
                for hh2 in range(2):
                    ps2 = psC.tile([128, 512], f32, tag="big")
                    for s in range(S):
                        nc.tensor.matmul(ps2[:], hc_t[s][:, tp * 128:(tp + 1) * 128],
                                         sdc_sb[:, s, hh2 * 512:(hh2 + 1) * 512],
                                         start=(s == 0), stop=(s == S - 1))
                    if hh2 == 0:
                        nc.vector.tensor_copy(so[:, hh2 * 512:(hh2 + 1) * 512],
                                              ps2[:])
                    else:
                        nc.scalar.activation(
                            so[:, hh2 * 512:(hh2 + 1) * 512], ps2[:], AF.Copy)
                nc.gpsimd.dma_start(sh_d.ap()[tp * 128:(tp + 1) * 128, :], so[:])

        emit_expert(0)
        wgs[2] = load_wgu(2)
        wds[2] = load_wd(2)
        emit_expert(1)
        wgs[3] = load_wgu(3)
        wds[3] = load_wd(3)
        emit_expert(2)
        emit_shared_gu()
        wgs[4] = load_wgu(4)
        wds[4] = load_wd(4)
        emit_expert(3)
        wgs[5] = load_wgu(5)
        wds[5] = load_wd(5)
        emit_expert(4)
        emit_shared_down()
        # last two slots: both gate_up weight streams load before either
        # down-proj weight, so only down7 + stores trail the DMA stream
        wgs[6] = load_wgu(6)
        wgs[7] = load_wgu(7)
        wds[6] = load_wd(6)
        wds[7] = load_wd(7)
        emit_expert(5)
        emit_expert(6)
        emit_expert(7)
    nc.compile()
    return nc


def _route(x, gate_w):
    """Exact replica of the reference's grouped top-k gating, on CPU jax."""
    import jax
    import jax.numpy as jnp
    cpu = jax.devices("cpu")[0]
    with jax.default_device(cpu):
        xj = jax.device_put(np.asarray(x), cpu)
        gj = jax.device_put(np.asarray(gate_w), cpu)
        logits = xj @ gj.T
        t = logits.shape[0]
        group_size = E // G
        group_logits = logits.reshape(t, G, group_size)
        gw, gi = jax.lax.top_k(group_logits, TOPK_GROUP)
        gw = gw.reshape(t, G * TOPK_GROUP)
        gi = gi.reshape(t, G * TOPK_GROUP)
        topk_w, ti = jax.lax.top_k(gw, K)
        sel_group = ti // TOPK_GROUP
        expert_in_group = jnp.take_along_axis(gi, ti, axis=1)
        topk_idx = sel_group * group_size + expert_in_group
        topk_w = topk_w / (topk_w.sum(axis=-1, keepdims=True) + 1e-20)
    return np.asarray(topk_idx), np.asarray(topk_w).astype(np.float32)


def _expert_np(xrows, w_gu_e, w_d_e):
    """Reference expert math in numpy fp32 (overflow fallback only)."""
    g = xrows @ w_gu_e
    a = g[:, :I]
    hidden = (a / (1.0 + np.exp(-a))) * g[:, I:]
    return hidden @ w_d_e


def kernel(x, gate_w, w_gu, w_d, s_gu, s_d):
    global _CACHED_NC, LAST_RESULTS
    x = np.ascontiguousarray(np.asarray(x, dtype=np.float32))
    gate_w = np.ascontiguousarray(np.asarray(gate_w, dtype=np.float32))
    w_gu = np.asarray(w_gu, dtype=np.float32)
    w_d = np.asarray(w_d, dtype=np.float32)
    s_gu = np.asarray(s_gu, dtype=np.float32)
    s_d = np.asarray(s_d, dtype=np.float32)

    topk_idx, topk_w = _route(x, gate_w)

    flat_e = topk_idx.ravel()
    flat_t = np.repeat(np.arange(T), K)
    flat_w = topk_w.ravel()
    order = np.argsort(flat_e, kind="stable")
    sorted_t = flat_t[order]
    sorted_w = flat_w[order]
    counts = np.bincount(flat_e, minlength=E)
    starts = np.zeros(E + 1, np.int64)
    np.cumsum(counts, out=starts[1:])

    # expert -> (core, slot) by descending count: global rank r -> core r%8,
    # slot r//8, so slot j sees only counts of rank band [8j, 8j+8)
    perm = np.argsort(-counts, kind="stable")

    xTb = np.ascontiguousarray(x.T.astype(BF))  # [H, T] bf16
    xg_flat = np.zeros((N_CORES, 128, XTOT), BF)
    overflow = []
    for r in range(E):
        e = int(perm[r])
        c, j = r % N_CORES, r // N_CORES
        cap = CAPS[j]
        n = int(counts[e])
        toks = sorted_t[starts[e]:starts[e] + n]
        nn = min(n, cap)
        # [128, HT, cap] view of this slot's span
        span = xg_flat[c, :, XOFF[j]:XOFF[j + 1]].reshape(128, HT, cap)
        span[:, :, :nn] = xTb[:, toks[:nn]].reshape(HT, 128, nn).transpose(1, 0, 2)
        if n > cap:
            ws = sorted_w[starts[e]:starts[e] + n]
            overflow.append((e, toks[cap:], ws[cap:]))

    # gate_up halves: q -> gate cols [q*256,+256) ++ up cols [512+q*256,+256)
    wgu_b = w_gu.astype(BF)
    wgu_hv = np.stack([
        np.concatenate([wgu_b[:, :, 0:256], wgu_b[:, :, 512:768]], axis=2),
        np.concatenate([wgu_b[:, :, 256:512], wgu_b[:, :, 768:1024]], axis=2),
    ], axis=1)                                                    # [E,2,H,512]
    wgu_s = wgu_hv.reshape(E, 2, HT, 128, 512).transpose(0, 1, 3, 2, 4)
    wd_s = w_d.astype(BF).reshape(E, IT, 128, H).transpose(0, 2, 1, 3)

    if _CACHED_NC is None:
        _CACHED_NC = _build_nc()
    nc = _CACHED_NC

    s_gu_b = s_gu.astype(BF)
    s_d_b = s_d.astype(BF)
    in_maps = []
    for c in range(N_CORES):
        eids = [int(perm[j * N_CORES + c]) for j in range(E_LOC)]
        tg, ig = c // 4, c % 4
        tsl = slice(tg * TSH, (tg + 1) * TSH)
        isl = slice(ig * ISHC, (ig + 1) * ISHC)
        # xt: this core's token half, partition-major
        xt_s = np.ascontiguousarray(
            xTb[:, tsl].reshape(HT, 128, TSH).transpose(1, 0, 2))
        # swh[s]: gate cols isl ++ up cols I+isl -> [H, 256] -> [128, HT, 256]
        swh = np.concatenate([s_gu_b[:, :, isl], s_gu_b[:, :, I:][:, :, isl]],
                             axis=2)                              # [S,H,2*ISHC]
        swh_s = swh.reshape(S, HT, 128, 2 * ISHC).transpose(0, 2, 1, 3)
        # sdc: [128 i-rows, S, H]
        sdc = np.ascontiguousarray(
            s_d_b[:, isl, :].transpose(1, 0, 2))                  # [128,S,H]
        in_maps.append({
            "xg": xg_flat[c],
            "wgu": wgu_s[eids],
            "wd": wd_s[eids],
            "xt": xt_s,
            "swh": swh_s,
            "sdc": sdc,
        })

    res = run_bass_kernel_spmd(nc, in_maps, list(range(N_CORES)), trace=_TRACE)
    LAST_RESULTS = res

    out = np.zeros((T, H), np.float32)
    for c in range(N_CORES):
        tg = c // 4
        out[tg * TSH:(tg + 1) * TSH] += res.results[c]["sh"].astype(np.float32)

    routed_rows = np.empty((T * K, H), np.float32)
    pos = 0
    tok_order = np.empty(T * K, np.int64)
    for r in range(E):
        e = int(perm[r])
        c, j = r % N_CORES, r // N_CORES
        cap = CAPS[j]
        n = min(int(counts[e]), cap)
        # [128, HT, cap] -> [H, cap]: row h = 128*chunk + p lives at [p, chunk]
        ye = (res.results[c]["y"][:, XOFF[j]:XOFF[j + 1]]
              .reshape(128, HT, cap).transpose(1, 0, 2)
              .reshape(H, cap)[:, :n].astype(np.float32))
        ws = sorted_w[starts[e]:starts[e] + n]
        routed_rows[pos:pos + n] = ye.T * ws[:, None]
        tok_order[pos:pos + n] = sorted_t[starts[e]:starts[e] + n]
        pos += n
    inv = np.argsort(tok_order[:pos], kind="stable")
    if pos == T * K:
        routed = routed_rows[inv].reshape(T, K, H).sum(axis=1)
        out += routed
    else:
        np.add.at(out, tok_order[:pos][inv], routed_rows[:pos][inv])

    for e, toks, ws in overflow:
        y_extra = _expert_np(x[toks], w_gu[e], w_d[e]) * ws[:, None]
        np.add.at(out, toks, y_extra)

    return out
